# revision 18
# baseline (speedup 1.0000x reference)
"""Morphological dilation (max-plus 3x3 depthwise conv) on 8 Trainium2 cores.

out[b,c,y,x] = max_{i,j in 3x3} ( x_pad[b,c,y+i,x+j] + se[c,i,j] ),
x: [16,64,256,256] f32, se: [64,3,3] f32, pad=1 with CVAL=-10000.

Sharding: pure data parallel. Core k takes batches {2k, 2k+1}; the 2*64
(batch,channel) pairs map onto the 128 SBUF partitions, so se[c,i,j] is a
per-partition scalar. Spatial dims live on the free axis.

Engine roles (measured: DVE fp16 tensor_scalar 4x when full-tile/aligned,
2x otherwise; tensor_tensor 2x; ACT Identity(x + per-partition bias) 1x at
1.2 GHz; GpSimd tensor ops ~10x below roofline -> unused for compute; PE
cannot max). Work split per 9-tap block: DVE does the 3 aligned j=0 adds
(tensor_scalar) + all 8 maxes (tensor_tensor); ACT does the 6 j=1/j=2 adds
into 4 rotating slot tiles. Slot g of tap k is freed by the fold TT that
reads it, 4 taps later - so ACT never waits on a whole block of folds
(the v4 3-slot ping-pong serialized fold->add->fold within each block).
A small `sp`-row tail of the last two ACT taps goes to DVE tensor_scalar
to balance DVE ~= ACT ~= 32 us per 24-row block.

Sync-wait budgets are 1 per instruction for the compute/DMA encodings used
here. Cross-engine handoffs rely on Tile auto-deps; instructions that would
carry >1 waits get same-engine Drain spills from a post-pass
(_split_excess_waits). Cheap 1-element gates keep the hot instructions at a
single wait: DVE memsets gw (acc-slot WAR vs the b-2 store) and gx (input
chunk), plus one ACT copy-gate ga per block (input chunk; reads se_t so its
only wait is the q0 DMA sem). x is fully SBUF-resident (one persistent
padded tile, 5 chunked loads serially chained on HWDGE queue 0); the 11
per-block stores use HWDGE queues 1-7 then 4 SWDGE queues.
"""

import os
import numpy as np

B, C, H, W = 16, 64, 256, 256
NCORES = 8
P = 128  # partitions = (B // NCORES) * C
CVAL = -10000.0
KH = KW = 3

_DTYPE = os.environ.get("DILATION_DTYPE", "f16")
_SP = int(os.environ.get("DILATION_SP", "1"))  # rows of a4/a5 done by DVE
_ROWS = int(os.environ.get("DILATION_ROWS", "18"))  # steady block rows
_NSLOTS = int(os.environ.get("DILATION_NSLOTS", "6"))  # rotating slot tiles
_DEFER = int(os.environ.get("DILATION_DEFER", "2"))  # folds deferred per block

_nc_cache = {}
LAST_RESULTS = None  # BassKernelResults of the most recent run (for profiling)

# instruction name -> forced HWDGE queue index (consulted by the patched
# TileClockTick._assign_tick during scheduling)
_FORCED_HW_QUEUE = {}
_ASSIGN_PATCHED = False

# tap order: g0,g1 = DVE tensor_scalar taps (j=0); g2..g7 = ACT taps.
# tap (0,0) is the acc init (extra, unslotted). Scalar index t = 3*i + j.
_DVE_TAPS = [(1, 0), (2, 0)]
_ACT_TAPS = [(0, 1), (1, 1), (2, 1), (0, 2), (1, 2), (2, 2)]


def _patch_queue_assignment():
    global _ASSIGN_PATCHED
    if _ASSIGN_PATCHED:
        return
    import concourse.tile_sem_assignment as tsa

    orig = tsa.TileClockTick._assign_tick

    def _assign_tick(self, inst):
        forced = _FORCED_HW_QUEUE.get(getattr(inst, "name", None))
        if forced is None:
            return orig(self, inst)
        save = self.next_hw_dma_idx
        self.next_hw_dma_idx = forced
        try:
            return orig(self, inst)
        finally:
            self.next_hw_dma_idx = save

    tsa.TileClockTick._assign_tick = _assign_tick
    _ASSIGN_PATCHED = True


def _split_excess_waits(nc, mybir, max_waits: int = 1):
    """Walrus's per-encoding sync-wait slots are scarce (1 for most ops used
    here). Hoist all but `max_waits` waits of any instruction onto freshly
    inserted same-engine Drain instructions placed right before it."""
    n = 0
    for bb in nc.main_func.blocks:
        insts = bb.instructions
        i = 0
        while i < len(insts):
            ins = insts[i]
            si = ins.sync_info
            if si is not None and len(si.on_wait) > max_waits:
                waits = list(si.on_wait)
                keep = waits[-max_waits:]
                spill = waits[:-max_waits]
                new_insts = []
                for w in spill:
                    d = mybir.InstDrain(name=f"wsplit-{n}", ins=[], outs=[])
                    n += 1
                    d.engine = ins.engine
                    d.sync_info = mybir.SyncInfo(on_wait=[w], on_update=[])
                    new_insts.append(d)
                ins.sync_info = mybir.SyncInfo(
                    on_wait=keep, on_update=list(si.on_update)
                )
                insts[i:i] = new_insts
                i += len(new_insts)
            i += 1
        bb.instructions = insts


def _build(
    dtype_tag: str,
    h: int = H,
    sp: int = _SP,
    rows_s: int = _ROWS,
    nslots: int = _NSLOTS,
    defer: int = _DEFER,
):
    import concourse.bass as bass
    import concourse.mybir as mybir
    from concourse.tile import TileContext, add_dep_helper

    _patch_queue_assignment()
    _FORCED_HW_QUEUE.clear()

    assert dtype_tag == "f16", "layout is fp16-only"
    dt = mybir.dt.float16
    f32 = mybir.dt.float32
    add = mybir.AluOpType.add
    vmax = mybir.AluOpType.max
    ident = mybir.ActivationFunctionType.Identity
    fcopy = mybir.ActivationFunctionType.Copy

    nc = bass.Bass(trn_type="TRN2", num_swdge_queues=4)
    # x arrives host-pre-padded to [P, h+2, W+2] with CVAL borders, so every
    # DMA chunk is one contiguous descriptor per partition (the old
    # 256-cols-into-258-pitch loads ran at ~218 GB/s, gating the ramp).
    x_d = nc.declare_dram_parameter("x", [P, h + 2, W + 2], dt, isOutput=False)
    se_d = nc.declare_dram_parameter("sep", [P, KH * KW], f32, isOutput=False)
    out_d = nc.declare_dram_parameter("out", [P, h, W], dt, isOutput=True)

    # 8-row first block starts compute as soon as the 10-row first chunk
    # lands; rows_s-row steady blocks; small last block drains fast.
    nsteady = (h - 16) // rows_s
    tail = h - 16 - nsteady * rows_s
    blocks = [8] + [rows_s] * nsteady + ([tail] if tail else []) + [8]
    assert sum(blocks) == h and all(b % 2 == 0 and b > 0 for b in blocks)
    maxrows = max(blocks)
    # SBUF: xt + (2 acc + nslots) block tiles must fit in ~208 KiB/partition
    assert (2 + nslots) * maxrows * 2 * W + (h + 2) * (W + 2) * 2 <= 212500
    # chunked loads of padded rows; block at y0 uses padded rows
    # [y0, y0+rows+2). Measured per-core load bw is ~200 GB/s (~0.35
    # us/row), so early chunks are small to pace arrivals to consumption.
    loads = [10, 24, 24, 24, 24, 48, 104]
    assert sum(loads) == h + 2

    with TileContext(nc) as tc:
        with (
            tc.tile_pool(name="const", bufs=1) as cpool,
            tc.tile_pool(name="xp", bufs=1) as xpool,
            tc.tile_pool(name="accp", bufs=2) as apool,
            tc.tile_pool(name="tmpp", bufs=1) as tpool,
        ):
            # se on its own queue so chunk0 starts immediately on q0
            se_t = cpool.tile([P, KH * KW], f32, name="se_t")
            se_dma = nc.sync.dma_start(out=se_t[:], in_=se_d[:])
            _FORCED_HW_QUEUE[se_dma.ins.name] = 7

            # One persistent padded-x tile: xt row t = padded-input row t.
            xt = xpool.tile([P, h + 2, W + 2], dt, name="xt")

            # Chunked contiguous loads, serially chained on HWDGE queue 0.
            load_dmas = []
            load_top = []  # last loaded padded row (exclusive) per chunk
            y0 = 0
            for rows in loads:
                ld = nc.sync.dma_start(
                    out=xt[:, y0 : y0 + rows, :],
                    in_=x_d[:, y0 : y0 + rows, :],
                )
                _FORCED_HW_QUEUE[ld.ins.name] = 0
                load_dmas.append(ld)
                y0 += rows
                load_top.append(y0)

            # tiny scratch targets for the DVE gates
            dve_scr = cpool.tile([P, 2 * len(blocks)], dt, name="dve_scr")
            act_scr = cpool.tile([P, 2], dt, name="act_scr")

            # nslots rotating slot tiles shared by the 8 non-init taps of each
            # block: global tap g -> slot g%nslots; the fold TT of tap g frees
            # the slot for tap g+nslots (Tile auto-deps enforce the WAR).
            slots = [
                tpool.tile([P, maxrows, W], dt, name=f"slot{i}")
                for i in range(nslots)
            ]

            out_dmas = []
            pending = []  # deferred (fold-emitter, store-emitter) of prev block
            y0 = 0
            for blk, rows in enumerate(blocks):
                # deepest load chunk this block needs (padded rows
                # [y0, y0+rows+2); the queue-0 chain covers earlier chunks)
                need_top = y0 + rows + 2
                ldi = next(i for i, top in enumerate(load_top) if top >= need_top)

                acc = apool.tile([P, rows, W], dt, name="acc")
                # DVE-side gates: gw absorbs the store whose acc slot this
                # block reuses, gx the input-chunk wait.
                if blk >= 2:
                    gw = nc.vector.memset(dve_scr[:, 2 * blk + 1 : 2 * blk + 2], 0.0)
                    add_dep_helper(gw.ins, out_dmas[blk - 2].ins, reason="acc WAR")
                gx = nc.vector.memset(dve_scr[:, 2 * blk : 2 * blk + 1], 0.0)
                add_dep_helper(gx.ins, load_dmas[ldi].ins, reason="input chunk")
                # ACT-side chunk gate; reads se_t so its single wait is the
                # q0 DMA sem (which also covers the se load itself).
                ga = nc.scalar.activation(act_scr[:, 0:1], se_t[:, 0:1], fcopy)
                add_dep_helper(ga.ins, load_dmas[ldi].ins, reason="input chunk/ACT")

                # acc init: tap (0,0), aligned full-tile tensor_scalar (4x)
                nc.vector.tensor_scalar(
                    acc[:],
                    xt[:, y0 : y0 + rows, 0:W],
                    se_t[:, 0:1],
                    None,
                    add,
                )

                # 8 non-init taps: g=0,1 are DVE tensor_scalar (j=0, full-tile
                # writes keep 4x); g=2..7 are ACT adds, the last two with an
                # sp-row DVE tail (aligned j=2) to balance the engines.
                spl = sp if rows >= 12 else 0
                gbase = 8 * blk

                def emit_tap(g, y0=y0, rows=rows, spl=spl, gbase=gbase):
                    st = slots[(gbase + g) % nslots]
                    if g < 2:
                        t_i, t_j = _DVE_TAPS[g]
                        sidx = 3 * t_i + t_j
                        nc.vector.tensor_scalar(
                            st[:, 0:rows, :] if rows < maxrows else st[:],
                            xt[:, y0 + t_i : y0 + t_i + rows, t_j : t_j + W],
                            se_t[:, sidx : sidx + 1],
                            None,
                            add,
                        )
                        return
                    k = g - 2
                    t_i, t_j = _ACT_TAPS[k]
                    sidx = 3 * t_i + t_j
                    arows = rows - spl if k >= 4 else rows
                    nc.scalar.activation(
                        st[:, 0:arows, :],
                        xt[:, y0 + t_i : y0 + t_i + arows, t_j : t_j + W],
                        ident,
                        bias=se_t[:, sidx : sidx + 1],
                    )
                    if arows < rows:
                        nc.vector.tensor_scalar(
                            st[:, arows:rows, :],
                            xt[:, y0 + t_i + arows : y0 + t_i + rows, t_j : t_j + W],
                            se_t[:, sidx : sidx + 1],
                            None,
                            add,
                        )

                def emit_fold(g, acc=acc, rows=rows, gbase=gbase):
                    nc.vector.tensor_tensor(
                        acc[:], acc[:], slots[(gbase + g) % nslots][:, 0:rows, :],
                        vmax,
                    )

                def emit_store(blk=blk, acc=acc, y0=y0, rows=rows):
                    # HWDGE queues 1..6 round-robin (7 is the se load); a
                    # queue's prior store finished ~6 block-periods earlier.
                    od = nc.sync.dma_start(
                        out=out_d[:, y0 : y0 + rows, :], in_=acc[:]
                    )
                    _FORCED_HW_QUEUE[od.ins.name] = 1 + (blk % 6)
                    out_dmas.append(od)

                # program order: tap g must follow fold g-nslots (slot reuse);
                # the last `defer` folds (+ store) move into the next block,
                # emitted after this block's independent DVE head (init+d1+d2)
                # so DVE has filler while ACT finishes the prev block's tail.
                n_head = 2  # d1,d2 slots were freed >= 2 folds before defer
                for g in range(n_head):
                    emit_tap(g)
                for emit in pending:
                    emit()
                pending = []
                for g in range(n_head, min(nslots, 8)):
                    emit_tap(g)
                for g in range(8):
                    emit_fold(g)
                    if g + nslots < 8:
                        emit_tap(g + nslots)
                    if g == 7 - defer:
                        break
                for g in range(8 - defer, 8):
                    pending.append(lambda g=g, f=emit_fold: f(g))
                pending.append(emit_store)
                if defer == 0:
                    for emit in pending:
                        emit()
                    pending = []
                y0 += rows
            for emit in pending:
                emit()

    _split_excess_waits(nc, mybir)
    return nc


def _get_nc():
    key = (_DTYPE, _SP)
    if key not in _nc_cache:
        _nc_cache[key] = _build(_DTYPE)
    return _nc_cache[key]


def kernel(x: np.ndarray, se: np.ndarray) -> np.ndarray:
    global LAST_RESULTS
    from concourse.bass_utils import run_bass_kernel_spmd

    np_dt = np.float16 if _DTYPE == "f16" else np.float32
    x = np.asarray(x)
    se = np.asarray(se)
    # host-side CVAL pre-pad -> every device DMA chunk is contiguous
    xs = np.full((NCORES, P, H + 2, W + 2), CVAL, dtype=np_dt)
    xs[:, :, 1 : H + 1, 1 : W + 1] = (
        np.ascontiguousarray(x).reshape(NCORES, P, H, W).astype(np_dt)
    )
    sep = np.ascontiguousarray(
        np.tile(np.asarray(se, np.float32).reshape(C, KH * KW), (P // C, 1))
    )

    nc = _get_nc()
    in_maps = [{"x": xs[k], "sep": sep} for k in range(NCORES)]
    trace = bool(os.environ.get("DILATION_TRACE"))
    kwargs = {}
    if trace:
        kwargs["trace"] = True
        tmpdir = os.environ.get("DILATION_TRACE_DIR")
        if tmpdir:
            kwargs["tmpdir"] = tmpdir
    res = run_bass_kernel_spmd(nc, in_maps, list(range(NCORES)), **kwargs)
    LAST_RESULTS = res
    out = np.stack([res.results[k]["out"] for k in range(NCORES)])
    return out.reshape(B, C, H, W).astype(np.float32)
